# revision 1
# baseline (speedup 1.0000x reference)
"""Multi-head self-attention with LoRA on 8 Trainium2 NeuronCores.

Sharding: core c -> (batch b = c//2, query-token-half = c%2). Each core
projects q/k/v only for its OWN 1024 tokens; the two cores of a batch
exchange k/v halves with pair-wise AllGather collectives (softmax is
key-permutation invariant, so no index fixup is needed). Each AllGather is
split into two half-collectives launched mid-projection so the exchange
hides under the remaining projection compute.

Layout/precision design (everything bf16; tolerance is 2e-2, fp8 was
measured at 4.4e-2 and rejected):
  - host folds LoRA into the weights (W' = W + 0.5*A@B), pre-transposes x,
    and pre-tiles the q/k/o weights into [do, p, n, f] stationary blocks
  - V is projected directly into natural [tok, dout] layout (xT tiles as
    the stationary operand) -> attention needs no PE transposes at all
  - v-bias is applied to the normalized attention output (per-partition add)
  - attention per (head, 512-query chunk): scores matmul pairs feed one
    1024-wide EXP on ACT; attn@v pairs interleave into the ACT-gated slots;
    softmax denominators are 4 col-tiled ones-matmul groups issued r-major
    so the four col-group streams run concurrently on the PE, combined and
    broadcast with a single selector matmul; reciprocal_approx_fast on DVE
  - matmuls are kept in contiguous same-PSUM-bank accumulation chains
    (interleaving across banks/col-groups costs ~100ns/matmul)
  - qT / ao stay SBUF-resident; weight tiles are preloaded several do-blocks
    ahead so phase transitions don't stall on DMA queue order
"""

import os
import numpy as np
import ml_dtypes

import concourse.bacc as bacc
import concourse.mybir as mybir
import concourse.tile as tile
from concourse.bass_utils import run_bass_kernel_spmd

F32 = mybir.dt.float32
BF16 = mybir.dt.bfloat16
F8 = mybir.dt.float8e4
DR = mybir.MatmulPerfMode.DoubleRow
AF = mybir.ActivationFunctionType

B, L, D = 4, 2048, 2048
H, HD, R = 16, 128, 16
SCALING = 0.5          # lora alpha / rank (folded into aT on host)
SCALE = HD ** -0.5     # attention score scale
P = 128                # partitions
NT = D // P            # 16 tiles along feature dims
QTOK = L // 2          # query tokens per core
CH = 512               # moving-dim chunk
NCORES = 8

COLTILE_DEN = False    # temp: aligned-combine variant pending
STRIDED_D4 = False     # strided-partition APs are rejected by the verifier

BF = ml_dtypes.bfloat16
F8NP = ml_dtypes.float8_e4m3
WSCALE = 64.0          # fp8 weight pre-scale (undone at psum evacuation)
NT2 = 8                # din-tile pairs for DoubleRow

_cache = {}


def _build():
    nc = bacc.Bacc()

    xT = nc.dram_tensor("xT", [D, QTOK], BF16, kind="ExternalInput")
    # tiled stationary weights [do, p, n, f] for q/k/o
    wq = nc.dram_tensor("wq", [NT, P, NT, P], BF16, kind="ExternalInput")
    wk = nc.dram_tensor("wk", [NT, P, NT, P], BF16, kind="ExternalInput")
    wo = nc.dram_tensor("wo", [NT, P, NT, P], BF16, kind="ExternalInput")
    wv = nc.dram_tensor("wv", [D, D], BF16, kind="ExternalInput")  # [din, dout]
    bias = {p: nc.dram_tensor(f"b{p}", [D], F32, kind="ExternalInput")
            for p in "qkvo"}
    yt = nc.dram_tensor("yt", [D, QTOK], BF16, kind="ExternalOutput")

    ones_d = nc.inline_tensor(np.ones((P, P), dtype=np.float32), name="ones_d")
    sel_np = np.zeros((P, P), dtype=np.float32)
    sel_np[0::32, :] = 1.0
    sel_d = nc.inline_tensor(sel_np, name="sel_d")

    def dma(out, in_):
        nc.sync.dma_start(out=out, in_=in_)

    def dma_g(out, in_):
        nc.sync.dma_start(out=out, in_=in_)

    with tile.TileContext(nc) as tc:
        with (
            tc.tile_pool(name="consts", bufs=1) as consts,
            tc.tile_pool(name="dram", bufs=1, space="DRAM") as dpool,
        ):
            # own-half projections exchanged pair-wise via AllGather, split
            # into two half-collectives each so they launch mid-projection
            # and attention can start on the first halves
            HD2 = D // 2
            kT_myA = dpool.tile([HD2, QTOK], BF16, tag="kT_myA")
            kT_myB = dpool.tile([HD2, QTOK], BF16, tag="kT_myB")
            v_myA = dpool.tile([NT // 2, P, HD2], BF16, tag="v_myA")
            v_myB = dpool.tile([NT // 2, P, HD2], BF16, tag="v_myB")
            kT_gA = dpool.tile([2, HD2, QTOK], BF16, tag="kT_gA")
            kT_gB = dpool.tile([2, HD2, QTOK], BF16, tag="kT_gB")
            v_gA = dpool.tile([2, NT // 2, P, HD2], BF16, tag="v_gA")
            v_gB = dpool.tile([2, NT // 2, P, HD2], BF16, tag="v_gB")
            RG = [[2 * i, 2 * i + 1] for i in range(NCORES // 2)]

            def allgather(src_ap, dst_ap):
                nc.gpsimd.collective_compute(
                    "AllGather", mybir.AluOpType.bypass, replica_groups=RG,
                    ins=[src_ap.opt()], outs=[dst_ap.opt()])

            with tc.tile_pool(name="qTp", bufs=1) as qTpool:
                qT_sb = qTpool.tile([P, NT, QTOK], BF16, tag="qT")  # 32KB/part

                # =============== Phase 1: xT load + z lora ===================
                with tc.tile_pool(name="xTp", bufs=1) as xTpool:
                    xT_sb = xTpool.tile([P, NT, QTOK], BF16, tag="xT")
                    for n in range(NT):
                        dma(xT_sb[:, n, :], xT[n * P:(n + 1) * P, :])

                    # persistent constants (emitted after xT so the
                    # critical-path input leads the DMA queues)
                    ones_f = consts.tile([P, P], F32, tag="ones_f")
                    dma(ones_f, ones_d[:, :])
                    ones = consts.tile([P, P], BF16, tag="ones")
                    nc.vector.tensor_copy(out=ones, in_=ones_f)
                    sel_f = consts.tile([P, P], F32, tag="sel_f")
                    dma(sel_f, sel_d[:, :])
                    sel = consts.tile([P, P], BF16, tag="sel")
                    nc.vector.tensor_copy(out=sel, in_=sel_f)
                    biasall = consts.tile([P, 4, NT], F32, tag="biasall")
                    for p in "qkvo":
                        dma(biasall[:, "qkvo".index(p), :],
                            bias[p][:].rearrange("(t p) -> p t", p=P))

                    # =============== Phase 2: K projection + AllGather =======
                    with (
                        tc.tile_pool(name="wvp", bufs=4) as wvpool,
                        tc.tile_pool(name="wqk", bufs=4) as wpool,
                        tc.tile_pool(name="oqk", bufs=3) as opool,
                        tc.tile_pool(name="pqk", bufs=4, space="PSUM") as pp,
                    ):
                        wk_tiles = []

                        def load_wk(do):
                            w_sb = wpool.tile([P, NT, P], BF16, tag="wqk",
                                              name=f"wk{do}")
                            dma_g(w_sb, wk[do, :, :, :])
                            wk_tiles.append(w_sb)

                        # first K weights ahead of the bulk wv preloads so the
                        # first matmuls aren't stuck behind 8MB in the queues
                        for do in range(2):
                            load_wk(do)
                        wv_tiles = []
                        for c0 in range(0, D, CH):
                            wv_sb = wvpool.tile([P, NT, CH], BF16, tag="wv",
                                                name=f"wv{c0}")
                            dma_g(wv_sb, wv[:, c0:c0 + CH]
                                  .rearrange("(n p) f -> p n f", p=P))
                            wv_tiles.append(wv_sb)
                        for do in range(NT):
                            if do + 2 < NT:
                                load_wk(do + 2)
                            kT_half = kT_myA if do < 8 else kT_myB
                            dl = (do % 8) * P
                            w_sb = wk_tiles[do]
                            for c0 in range(0, QTOK, CH):
                                ps = pp.tile([P, CH], F32, tag="pqk")
                                for ki in range(NT):
                                    nc.tensor.matmul(ps, w_sb[:, ki, :],
                                                     xT_sb[:, ki, c0:c0 + CH],
                                                     start=(ki == 0),
                                                     stop=(ki == NT - 1))
                                o_sb = opool.tile([P, CH], BF16, tag="oqk")
                                nc.vector.tensor_scalar_add(
                                    o_sb, ps, biasall[:, 1, do:do + 1])
                                dma(kT_half[dl:dl + P, c0:c0 + CH], o_sb)
                            if do == 7:
                                allgather(kT_myA[:, :], kT_gA[:, :, :])
                        allgather(kT_myB[:, :], kT_gB[:, :, :])

                    # =============== Phase 3: V natural projection ===========
                        wq_tiles = []

                        def load_wq(do):
                            w_sb = wpool.tile([P, NT, P], BF16, tag="wqk",
                                              name=f"wq{do}")
                            dma_g(w_sb, wq[do, :, :, :])
                            wq_tiles.append(w_sb)

                        for do in range(4):
                            load_wq(do)
                        with (
                            tc.tile_pool(name="vo", bufs=3) as vopool,
                            tc.tile_pool(name="pv", bufs=4, space="PSUM") as pv,
                        ):
                            for c0 in range(0, D, CH):
                                v_half = v_myA if c0 < HD2 else v_myB
                                cl = c0 % HD2
                                wv_sb = wv_tiles[c0 // CH]
                                for tb in range(NT // 2):
                                    ps = pv.tile([P, CH], F32, tag="pv")
                                    for n in range(NT):
                                        nc.tensor.matmul(
                                            ps, xT_sb[:, n, tb * P:(tb + 1) * P],
                                            wv_sb[:, n, :],
                                            start=(n == 0), stop=(n == NT - 1))
                                    v_sb = vopool.tile([P, CH], BF16, tag="v_sb")
                                    nc.vector.tensor_copy(out=v_sb, in_=ps)
                                    dma(v_half[tb, :, cl:cl + CH], v_sb)
                                if c0 == HD2 - CH:
                                    allgather(v_myA[:, :, :], v_gA[:, :, :, :])
                            allgather(v_myB[:, :, :], v_gB[:, :, :, :])

                    # =============== Phase 4b: Q projection ==================
                        for do in range(NT):
                            if do + 4 < NT:
                                load_wq(do + 4)
                            w_sb = wq_tiles[do]
                            for c0 in range(0, QTOK, CH):
                                ps = pp.tile([P, CH], F32, tag="pqk")
                                for ki in range(NT):
                                    nc.tensor.matmul(ps, w_sb[:, ki, :],
                                                     xT_sb[:, ki, c0:c0 + CH],
                                                     start=(ki == 0),
                                                     stop=(ki == NT - 1))
                                nc.vector.tensor_scalar_add(
                                    qT_sb[:, do, c0:c0 + CH], ps,
                                    biasall[:, 0, do:do + 1])

                # =============== Phase 4: attention per head =================
                with tc.tile_pool(name="aop", bufs=1) as aopool:
                    ao = aopool.tile([P, H, QTOK], BF16, tag="ao")  # 32KB/part

                    with (
                        tc.tile_pool(name="kh", bufs=4) as khpool,
                        tc.tile_pool(name="vh", bufs=4) as vhpool,
                        tc.tile_pool(name="ex", bufs=2) as expool,
                        tc.tile_pool(name="asb", bufs=3) as asbpool,
                        tc.tile_pool(name="ps_s", bufs=2,
                                     space="PSUM") as ps_spool,
                        tc.tile_pool(name="d128p", bufs=1) as d128pool,
                        tc.tile_pool(name="ps_dr", bufs=1,
                                     space="PSUM") as ps_drpool,
                        tc.tile_pool(name="ps_o", bufs=2,
                                     space="PSUM") as ps_opool,
                    ):
                        d128 = d128pool.tile([P, CH], BF16, tag="d128")
                        nc.vector.memset(d128, 0.0)
                        for hh in range(H):
                            kT_gh = kT_gA if hh < 8 else kT_gB
                            v_gh = v_gA if hh < 8 else v_gB
                            hl = (hh % 8) * P
                            kT_h = khpool.tile([P, L], BF16, tag="kT_h")
                            dma(kT_h[:, 0:QTOK], kT_gh[0, hl:hl + P, :])
                            dma(kT_h[:, QTOK:L], kT_gh[1, hl:hl + P, :])
                            v_h = vhpool.tile([P, NT, P], BF16, tag="v_h")
                            dma(v_h[:, 0:NT // 2, :],
                                v_gh[0, :, :, hl:hl + P]
                                .rearrange("k p f -> p k f"))
                            dma(v_h[:, NT // 2:NT, :],
                                v_gh[1, :, :, hl:hl + P]
                                .rearrange("k p f -> p k f"))

                            for c0 in range(0, QTOK, CH):
                                ex = expool.tile([P, NT, CH], BF16, tag="ex")
                                ps_o = ps_opool.tile([P, CH], F32, tag="ps_o")
                                # scores -> wide exp pipelined in kt pairs; the
                                # ACT-gated idle slots are filled with attn@v
                                # pairs lagging two exp-pairs behind
                                def av_pair(kt2a):
                                    for kt in (kt2a, kt2a + 1):
                                        nc.tensor.matmul(
                                            ps_o, v_h[:, kt, :], ex[:, kt, :],
                                            start=(kt == 0),
                                            stop=(kt == NT - 1))
                                for kt2 in range(0, NT, 2):
                                    ps_s = ps_spool.tile([P, 2 * CH], F32,
                                                         tag="ps_s")
                                    for j in range(2):
                                        kt = kt2 + j
                                        nc.tensor.matmul(
                                            ps_s[:, j * CH:(j + 1) * CH],
                                            kT_h[:, kt * P:(kt + 1) * P],
                                            qT_sb[:, hh, c0:c0 + CH],
                                            start=True, stop=True)
                                    nc.scalar.activation(
                                        ex[:, kt2:kt2 + 2, :], ps_s,
                                        AF.Exp, scale=SCALE)
                                    if kt2 >= 4:
                                        av_pair(kt2 - 4)
                                for kt2 in range(NT - 4, NT, 2):
                                    av_pair(kt2)
                                # denominators: 4 col-groups r-major so the
                                # 4 streams run concurrently; per-group
                                # start/stop (has_written is per-element)
                                ps_d = ps_drpool.tile([P, CH], F32, tag="ps_d")
                                for r in range(4):
                                    for g in range(4):
                                        nc.tensor.matmul(
                                            ps_d[32 * g:32 * g + 1, :],
                                            ones[:, 0:1], ex[:, 4 * g + r, :],
                                            start=(r == 0), stop=(r == 3),
                                            tile_position=(0, 32 * g))
                                for g in range(4):
                                    nc.vector.tensor_copy(
                                        out=d128[32 * g:32 * g + 1, :],
                                        in_=ps_d[32 * g:32 * g + 1, :])
                                ps_r = ps_drpool.tile([P, CH], F32, tag="ps_r")
                                nc.tensor.matmul(ps_r, sel, d128,
                                                 start=True, stop=True)
                                rb = asbpool.tile([P, CH], F32, tag="rb")
                                nc.vector.reciprocal_approx_fast(rb, ps_r)
                                tmp = asbpool.tile([P, CH], F32, tag="tmp")
                                nc.vector.tensor_mul(tmp, ps_o, rb)
                                nc.vector.tensor_scalar_add(
                                    ao[:, hh, c0:c0 + CH], tmp,
                                    biasall[:, 2, hh:hh + 1])

                    # =============== Phase 5: O projection ===================
                    with (
                        tc.tile_pool(name="wop", bufs=4) as wopool,
                        tc.tile_pool(name="oo", bufs=3) as oopool,
                        tc.tile_pool(name="po", bufs=4, space="PSUM") as po,
                    ):
                        for do in range(NT):
                            wo_sb = wopool.tile([P, NT, P], BF16, tag="wo")
                            dma_g(wo_sb, wo[do, :, :, :])
                            for c0 in range(0, QTOK, CH):
                                ps = po.tile([P, CH], F32, tag="po")
                                for ki in range(NT):
                                    nc.tensor.matmul(ps, wo_sb[:, ki, :],
                                                     ao[:, ki, c0:c0 + CH],
                                                     start=(ki == 0),
                                                     stop=(ki == NT - 1))
                                o_sb = oopool.tile([P, CH], BF16, tag="oo")
                                nc.vector.tensor_scalar_add(
                                    o_sb, ps, biasall[:, 3, do:do + 1])
                                dma(yt[do * P:(do + 1) * P, c0:c0 + CH], o_sb)

    nc.compile()
    return nc


def _prep_shared(inp):
    sh = {}
    for p in "qkvo":
        # fold LoRA into the weight: W' = W + SCALING * A @ B
        W = (inp[f"W{p}"].astype(np.float64)
             + SCALING * inp[f"A{p}"].astype(np.float64)
             @ inp[f"B{p}"].astype(np.float64))
        if p == "v":
            sh["wv"] = np.ascontiguousarray(W.T).astype(BF)
        else:
            # [do, p, n, f]: W.T[n*128+p, do*128+f]
            t = W.T.reshape(NT, P, NT, P).transpose(2, 1, 0, 3)
            sh[f"w{p}"] = np.ascontiguousarray(t).astype(BF)
        sh[f"b{p}"] = inp[f"b{p}"].astype(np.float32)
    return sh


def kernel(**inputs):
    inp = {k: np.asarray(v, dtype=np.float32) for k, v in inputs.items()}
    x = inp["x"]

    if "nc" not in _cache:
        _cache["nc"] = _build()
    nc = _cache["nc"]

    shared = _prep_shared(inp)

    in_maps = []
    for c in range(NCORES):
        b, hf = c // 2, c % 2
        m = dict(shared)
        m["xT"] = np.ascontiguousarray(
            x[b, hf * QTOK:(hf + 1) * QTOK].T).astype(BF)
        in_maps.append(m)

    trace = bool(int(os.environ.get("KERNEL_TRACE", "0")))
    res = run_bass_kernel_spmd(nc, in_maps, list(range(NCORES)), trace=trace)
    _cache["last_exec_time_ns"] = res.exec_time_ns
    _cache["last_result"] = res

    y = np.empty((B, L, D), dtype=np.float32)
    for c in range(NCORES):
        b, hf = c // 2, c % 2
        y[b, hf * QTOK:(hf + 1) * QTOK, :] = \
            res.results[c]["yt"].T.astype(np.float32)
    return y

